# revision 1
# baseline (speedup 1.0000x reference)
"""GCN (4-layer, PyG GCNConv semantics) on 8 Trainium2 NeuronCores.

Sharding: nodes partitioned into 8 contiguous blocks (graph-parallel, per the
halo-exchange hint); edges assigned to the core owning their dst. Each layer:
local dense matmul g=(h*dis)@W, AllGather of g across cores, then per-core
gather of src rows (indirect DMA) + segment-sum via one-hot selection matmuls
accumulating in PSUM. Weights/question embeddings replicated.

Self-contained: hardcodes the problem shapes (N=100000, E=1600000, B=64, D=128).
"""

import numpy as np

import concourse.bacc as bacc
import concourse.bass as bass
import concourse.mybir as mybir
import concourse.tile as tile

N = 100000
E = 1600000
B = 64
D = 128
NC = 8
NPC = N // NC          # real nodes per core (12500)
NT = (NPC + 127) // 128  # node tiles per core (98)
NB = NT * 128          # padded node slots per core (12544)
PAD_ROW = NPC          # a padded slot on rank 0: dis=0 there -> g row is 0
KG = 4                 # gather-chunks per indirect DMA

F32 = mybir.dt.float32
I32 = mybir.dt.int32
# dtype of the all-gathered g table / gathered edge rows / selection matrices
GDT = mybir.dt.float16
GNP = np.float16

_CACHE = {}


def _preprocess(x, edge_index, batch, question_embedding,
                w0, b0, w1, b1, w2, b2, w3, b3,
                fc0_w, fc0_b, fc1_w, fc1_b, fc2_w, fc2_b):
    src = np.asarray(edge_index[0], dtype=np.int64)
    dst = np.asarray(edge_index[1], dtype=np.int64)
    x = np.asarray(x, dtype=np.float32)
    batch = np.asarray(batch, dtype=np.int64)

    deg = (np.bincount(dst, minlength=N) + 1).astype(np.float32)
    dis = deg ** -0.5  # f32, matches reference closely enough

    core = dst // NPC
    rem = dst - core * NPC
    tile_of = rem >> 7
    key = core * NT + tile_of
    order = np.argsort(key, kind="stable")
    key_s = key[order]
    src_s = src[order]
    slot_s = (rem & 127)[order]

    cnt = np.bincount(key, minlength=NC * NT).reshape(NC, NT)
    cpt = np.maximum(np.ceil(cnt / 128).astype(np.int64).max(axis=0), 1)  # [NT]
    off = np.zeros(NT, dtype=np.int64)
    off[1:] = np.cumsum(cpt)[:-1]
    TC = int(cpt.sum())

    # remapped source row in the all-gathered table
    src_remap = ((src_s // NPC) * NB + (src_s % NPC)).astype(np.int32)

    idx_big = np.full((NC, 128, TC), PAD_ROW, dtype=np.int32)
    dstf_big = np.zeros((NC, 128, TC), dtype=np.float32)

    # position of each edge within its (core, tile) bucket
    starts = np.zeros(NC * NT, dtype=np.int64)
    starts[1:] = np.cumsum(cnt.reshape(-1))[:-1]
    pos = np.arange(len(key_s)) - starts[key_s]
    chunk = pos >> 7
    p = pos & 127
    col = off[key_s % NT] + chunk
    c_of = key_s // NT
    idx_big[c_of, p, col] = src_remap
    dstf_big[c_of, p, col] = slot_s.astype(np.float32)

    # per-core padded node data
    dis_pad = np.zeros((NC, NB), dtype=np.float32)
    batch_pad = np.zeros((NC, NB), dtype=np.int32)
    xds = x * dis[:, None]
    xdT = np.zeros((NC, 128, NB), dtype=np.float32)
    for c in range(NC):
        blk = slice(c * NPC, (c + 1) * NPC)
        dis_pad[c, :NPC] = dis[blk]
        batch_pad[c, :NPC] = batch[blk]
        xdT[c, :, :NPC] = xds[blk].T
    discolT = dis_pad.reshape(NC, NT, 128).transpose(0, 2, 1).copy()   # [NC,128,NT]
    batchT = batch_pad.reshape(NC, NT, 128).transpose(0, 2, 1).copy()  # [NC,128,NT]

    # weights (replicated)
    w_all = np.concatenate([np.asarray(w, np.float32) for w in (w0, w1, w2, w3)], axis=0)  # [512,128]
    brep1 = np.concatenate(
        [np.tile(np.asarray(b, np.float32)[None, :] + 1.0, (128, 1)) for b in (b0, b1, b2, b3)],
        axis=0,
    )  # [512,128]
    fc0_w = np.asarray(fc0_w, np.float32)   # [768,128]
    fc0w_packed = np.concatenate([fc0_w[j * 128:(j + 1) * 128, :] for j in range(6)], axis=1)  # [128,768]
    qT = np.asarray(question_embedding, np.float32).T  # [768, 64]
    qembT_packed = np.concatenate([qT[j * 128:(j + 1) * 128, :] for j in range(6)], axis=1)  # [128,384]
    fc0_b1_rep = np.tile(np.asarray(fc0_b, np.float32)[None, :] + 1.0, (128, 1))
    fc1a = np.ascontiguousarray(np.asarray(fc1_w, np.float32)[:128, :])
    fc1b = np.ascontiguousarray(np.asarray(fc1_w, np.float32)[128:, :])
    fc1_b1_rep = np.tile(np.asarray(fc1_b, np.float32)[None, :] + 1.0, (128, 1))
    fc2_w = np.asarray(fc2_w, np.float32)
    fc2_b_rep = np.tile(np.asarray(fc2_b, np.float32)[None, :], (128, 1))
    iota = np.tile(np.arange(128, dtype=np.float32)[None, :], (128, 1))
    ident = np.eye(128, dtype=np.float32)
    ident_g = np.eye(128, dtype=GNP)

    shared = dict(
        w_all=w_all, brep1=brep1, fc0w_packed=fc0w_packed, qembT_packed=qembT_packed,
        fc0_b1_rep=fc0_b1_rep, fc1a=fc1a, fc1b=fc1b, fc1_b1_rep=fc1_b1_rep,
        fc2_w=fc2_w, fc2_b_rep=fc2_b_rep, iota=iota, ident=ident, ident_g=ident_g,
    )
    in_maps = []
    for c in range(NC):
        m = dict(shared)
        m.update(
            xdT=np.ascontiguousarray(xdT[c]),
            discolT=np.ascontiguousarray(discolT[c]),
            batchT=np.ascontiguousarray(batchT[c]),
            idx_big=np.ascontiguousarray(idx_big[c]),
            dstf_big=np.ascontiguousarray(dstf_big[c]),
        )
        in_maps.append(m)
    return in_maps, [int(v) for v in cpt], TC


def _elu_chain(nc, sbuf, psum_or_sbuf, dis_scale, brep1_t, neg1, extra_add=None):
    """h = ELU(psum * dis + b) computed as:
    t = psum*dis; u1 = t + (b+1); m = min(u1,1); e = exp(m-1); v = max(u1,e); h = v-1 (+extra).
    Returns the h tile (f32 sbuf)."""
    t = sbuf.tile([128, D], F32, tag="elu_t")
    nc.scalar.activation(t[:], psum_or_sbuf, mybir.ActivationFunctionType.Copy, scale=dis_scale)
    u1 = sbuf.tile([128, D], F32, tag="elu_u1")
    nc.vector.tensor_add(u1[:], t[:], brep1_t)
    m = sbuf.tile([128, D], F32, tag="elu_m")
    nc.vector.tensor_scalar(m[:], u1[:], 1.0, None, mybir.AluOpType.min)
    e = sbuf.tile([128, D], F32, tag="elu_e")
    nc.scalar.activation(e[:], m[:], mybir.ActivationFunctionType.Exp, bias=neg1)
    v = sbuf.tile([128, D], F32, tag="elu_v")
    nc.vector.tensor_tensor(v[:], u1[:], e[:], op=mybir.AluOpType.max)
    h = sbuf.tile([128, D], F32, tag="elu_h")
    nc.scalar.activation(h[:], v[:], mybir.ActivationFunctionType.Copy, bias=-1.0)
    if extra_add is not None:
        h2 = sbuf.tile([128, D], F32, tag="elu_h2")
        nc.vector.tensor_add(h2[:], h[:], extra_add)
        return h2
    return h


def _build(cpt, TC):
    import os
    nt_use = int(os.environ.get("NT_DEBUG", NT))
    layer_reps = int(os.environ.get("LAYER_REPS", 1))
    no_ag = bool(int(os.environ.get("NO_AG", "0")))
    no_gather = bool(int(os.environ.get("NO_GATHER", "0")))
    nc = bacc.Bacc("TRN2", target_bir_lowering=False, debug=False, num_devices=NC)

    # per-core inputs
    xdT = nc.dram_tensor("xdT", [128, NB], F32, kind="ExternalInput")
    discolT = nc.dram_tensor("discolT", [128, NT], F32, kind="ExternalInput")
    batchT = nc.dram_tensor("batchT", [128, NT], I32, kind="ExternalInput")
    idx_big = nc.dram_tensor("idx_big", [128, TC], I32, kind="ExternalInput")
    dstf_big = nc.dram_tensor("dstf_big", [128, TC], F32, kind="ExternalInput")
    # replicated inputs
    w_all = nc.dram_tensor("w_all", [512, D], F32, kind="ExternalInput")
    brep1 = nc.dram_tensor("brep1", [512, D], F32, kind="ExternalInput")
    fc0w_packed = nc.dram_tensor("fc0w_packed", [128, 768], F32, kind="ExternalInput")
    qembT_packed = nc.dram_tensor("qembT_packed", [128, 384], F32, kind="ExternalInput")
    fc0_b1_rep = nc.dram_tensor("fc0_b1_rep", [128, D], F32, kind="ExternalInput")
    fc1a = nc.dram_tensor("fc1a", [128, D], F32, kind="ExternalInput")
    fc1b = nc.dram_tensor("fc1b", [128, D], F32, kind="ExternalInput")
    fc1_b1_rep = nc.dram_tensor("fc1_b1_rep", [128, D], F32, kind="ExternalInput")
    fc2_w = nc.dram_tensor("fc2_w", [128, D], F32, kind="ExternalInput")
    fc2_b_rep = nc.dram_tensor("fc2_b_rep", [128, D], F32, kind="ExternalInput")
    iota_in = nc.dram_tensor("iota", [128, 128], F32, kind="ExternalInput")
    ident_in = nc.dram_tensor("ident", [128, 128], F32, kind="ExternalInput")
    ident_g_in = nc.dram_tensor("ident_g", [128, 128], GDT, kind="ExternalInput")

    out = nc.dram_tensor("out", [NB, D], F32, kind="ExternalOutput")

    with tile.TileContext(nc) as tc:
        with (
            tc.tile_pool(name="const", bufs=1) as cpool,
            tc.tile_pool(name="wpool", bufs=2) as wpool,
            tc.tile_pool(name="dense", bufs=4) as dense,
            tc.tile_pool(name="gather", bufs=24) as gpool,
            tc.tile_pool(name="sel", bufs=8) as selpool,
            tc.tile_pool(name="fin", bufs=3) as fin,
            tc.tile_pool(name="psum", bufs=2, space="PSUM") as psum,
            tc.tile_pool(name="apsum", bufs=3, space="PSUM") as apsum,
            tc.tile_pool(name="dram", bufs=1, space="DRAM") as dram,
        ):
            ag_in = dram.tile([NB, D], GDT, tag="ag_in")
            g_fulls = [dram.tile([NC * NB, D], GDT, addr_space="Shared",
                                 tag=f"g_full{l}", name=f"g_full{l}")
                       for l in range(4 * layer_reps)]
            hdT_a = dram.tile([128, NB], F32, tag="hdT_a")
            hdT_b = dram.tile([128, NB], F32, tag="hdT_b")
            h2_buf = dram.tile([NB, D], F32, tag="h2_buf")
            qq_dram = dram.tile([B, D], F32, tag="qq_dram")

            # constants
            iota_t = cpool.tile([128, 128], F32)
            nc.sync.dma_start(out=iota_t[:], in_=iota_in[:, :])
            ident_t = cpool.tile([128, 128], F32)
            nc.sync.dma_start(out=ident_t[:], in_=ident_in[:, :])
            identg_t = cpool.tile([128, 128], GDT)
            nc.sync.dma_start(out=identg_t[:], in_=ident_g_in[:, :])
            neg1 = cpool.tile([128, 1], F32)
            nc.vector.memset(neg1[:], -1.0)
            dis_t = cpool.tile([128, NT], F32)
            nc.sync.dma_start(out=dis_t[:], in_=discolT[:, :])
            batch_t = cpool.tile([128, NT], I32)
            nc.sync.dma_start(out=batch_t[:], in_=batchT[:, :])

            # ---------------- question head (replicated) ----------------
            qembT_t = cpool.tile([128, 384], F32)
            nc.sync.dma_start(out=qembT_t[:], in_=qembT_packed[:, :])
            fc0w_t = cpool.tile([128, 768], F32)
            nc.sync.dma_start(out=fc0w_t[:], in_=fc0w_packed[:, :])
            fc0b1_t = cpool.tile([128, D], F32)
            nc.sync.dma_start(out=fc0b1_t[:], in_=fc0_b1_rep[:, :])
            fc1b_t = cpool.tile([128, D], F32)
            nc.sync.dma_start(out=fc1b_t[:], in_=fc1b[:, :])
            fc1b1_t = cpool.tile([128, D], F32)
            nc.sync.dma_start(out=fc1b1_t[:], in_=fc1_b1_rep[:, :])

            q0_ps = psum.tile([64, D], F32, tag="dmm")
            for j in range(6):
                nc.tensor.matmul(
                    q0_ps[:], qembT_t[:, j * 64:(j + 1) * 64], fc0w_t[:, j * 128:(j + 1) * 128],
                    start=(j == 0), stop=(j == 5),
                )
            # ELU on [64,128]
            qu1 = cpool.tile([64, D], F32)
            nc.vector.tensor_add(qu1[:], q0_ps[:], fc0b1_t[:64, :])
            qm = cpool.tile([64, D], F32)
            nc.vector.tensor_scalar(qm[:], qu1[:], 1.0, None, mybir.AluOpType.min)
            qe = cpool.tile([64, D], F32)
            nc.scalar.activation(qe[:], qm[:], mybir.ActivationFunctionType.Exp, bias=neg1[:64, :1])
            qv = cpool.tile([64, D], F32)
            nc.vector.tensor_tensor(qv[:], qu1[:], qe[:], op=mybir.AluOpType.max)
            q_t = cpool.tile([64, D], F32)
            nc.scalar.activation(q_t[:], qv[:], mybir.ActivationFunctionType.Copy, bias=-1.0)
            # qT
            qT_ps = psum.tile([128, 64], F32, tag="tp")
            nc.tensor.transpose(qT_ps[:], q_t[:], ident_t[:64, :64])
            qT_t = cpool.tile([128, 64], F32)
            nc.vector.tensor_copy(qT_t[:], qT_ps[:])
            qq_ps = psum.tile([64, D], F32, tag="dmm")
            nc.tensor.matmul(qq_ps[:], qT_t[:], fc1b_t[:], start=True, stop=True)
            qq_t = cpool.tile([64, D], F32)
            nc.vector.tensor_add(qq_t[:], qq_ps[:], fc1b1_t[:64, :])
            nc.sync.dma_start(out=qq_dram[:, :], in_=qq_t[:])

            # ---------------- GCN layers ----------------
            dense_src = [xdT, hdT_a, hdT_b, hdT_a] * layer_reps
            agg_dstT = [hdT_a, hdT_b, hdT_a, hdT_b] * layer_reps
            for rep in range(layer_reps - 1):
                dense_src[4 * (rep + 1)] = hdT_b  # later reps read the prev rep's h4T
            for glayer in range(4 * layer_reps):
                layer = glayer % 4
                w_t = wpool.tile([128, D], F32, tag="w")
                nc.sync.dma_start(out=w_t[:], in_=w_all[layer * 128:(layer + 1) * 128, :])
                b1_t = wpool.tile([128, D], F32, tag="b")
                nc.sync.dma_start(out=b1_t[:], in_=brep1[layer * 128:(layer + 1) * 128, :])

                # dense: g = (h*dis) @ w, stored to ag_in (GDT)
                hsrc = dense_src[glayer]
                for t in range(nt_use):
                    hdT_t = dense.tile([128, 128], F32, tag="hdT_in")
                    nc.sync.dma_start(out=hdT_t[:], in_=hsrc[:, t * 128:(t + 1) * 128])
                    g_ps = psum.tile([128, D], F32, tag="dmm")
                    nc.tensor.matmul(g_ps[:], hdT_t[:], w_t[:], start=True, stop=True)
                    g_sb = dense.tile([128, D], GDT, tag="g_out")
                    nc.scalar.copy(g_sb[:], g_ps[:])
                    nc.sync.dma_start(out=ag_in[t * 128:(t + 1) * 128, :], in_=g_sb[:])

                g_full = g_fulls[glayer]
                if not no_ag:
                    nc.gpsimd.collective_compute(
                        "AllGather",
                        mybir.AluOpType.bypass,
                        replica_groups=[list(range(NC))],
                        ins=[ag_in.opt()],
                        outs=[g_full.opt()],
                    )

                # aggregation per tile
                hdst = agg_dstT[glayer]
                for t in range(nt_use):
                    ct = cpt[t]
                    off_t = sum(cpt[:t])
                    idx_t = selpool.tile([128, ct], I32, tag="idx")
                    nc.sync.dma_start(out=idx_t[:], in_=idx_big[:, off_t:off_t + ct])
                    dstf_t = selpool.tile([128, ct], F32, tag="dstf")
                    nc.sync.dma_start(out=dstf_t[:], in_=dstf_big[:, off_t:off_t + ct])

                    gbufs = []
                    for k in range(ct):
                        gb = gpool.tile([128, D], GDT, tag="gbuf")
                        if not no_gather:
                            gi = nc.gpsimd.indirect_dma_start(
                                out=gb[:],
                                out_offset=None,
                                in_=g_full[:],
                                in_offset=bass.IndirectOffsetOnAxis(
                                    ap=idx_t[:, k:k + 1], axis=0),
                            )
                        gbufs.append(gb)

                    agg_ps = apsum.tile([128, D], F32, tag="agg")
                    for k in range(ct):
                        sel = selpool.tile([128, 128], GDT, tag="sel")
                        nc.vector.tensor_tensor(
                            sel[:], dstf_t[:, k:k + 1].to_broadcast([128, 128]), iota_t[:],
                            op=mybir.AluOpType.is_equal,
                        )
                        nc.tensor.matmul(
                            agg_ps[:], sel[:], gbufs[k][:],
                            start=(k == 0), stop=False,
                        )
                    # self-loop: += I @ g_self
                    gself = fin.tile([128, D], GDT, tag="gself")
                    nc.sync.dma_start(out=gself[:], in_=ag_in[t * 128:(t + 1) * 128, :])
                    nc.tensor.matmul(agg_ps[:], identg_t[:], gself[:], start=False, stop=True)

                    extra = None
                    if layer == 3:
                        extra_t = fin.tile([128, D], F32, tag="h2in")
                        nc.sync.dma_start(out=extra_t[:], in_=h2_buf[t * 128:(t + 1) * 128, :])
                        extra = extra_t[:]
                    h_t = _elu_chain(nc, fin, agg_ps[:], dis_t[:, t:t + 1], b1_t[:], neg1[:, :1], extra_add=extra)

                    if layer == 1:
                        nc.sync.dma_start(out=h2_buf[t * 128:(t + 1) * 128, :], in_=h_t[:])
                    if layer < 3:
                        hd = fin.tile([128, D], F32, tag="hd")
                        nc.vector.tensor_scalar(
                            hd[:], h_t[:], dis_t[:, t:t + 1], None, mybir.AluOpType.mult)
                        tp_ps = psum.tile([128, 128], F32, tag="tp")
                        nc.tensor.transpose(tp_ps[:], hd[:], ident_t[:])
                    else:
                        tp_ps = psum.tile([128, 128], F32, tag="tp")
                        nc.tensor.transpose(tp_ps[:], h_t[:], ident_t[:])
                    hdT_o = fin.tile([128, 128], F32, tag="hdT_out")
                    nc.scalar.copy(hdT_o[:], tp_ps[:])
                    nc.sync.dma_start(out=hdst[:, t * 128:(t + 1) * 128], in_=hdT_o[:])

            # ---------------- MLP head ----------------
            fc1a_t = cpool.tile([128, D], F32)
            nc.sync.dma_start(out=fc1a_t[:], in_=fc1a[:, :])
            fc2w_t = cpool.tile([128, D], F32)
            nc.sync.dma_start(out=fc2w_t[:], in_=fc2_w[:, :])
            fc2b_t = cpool.tile([128, D], F32)
            nc.sync.dma_start(out=fc2b_t[:], in_=fc2_b_rep[:, :])

            h4T = agg_dstT[3]
            for t in range(nt_use):
                h4T_t = dense.tile([128, 128], F32, tag="hdT_in")
                nc.sync.dma_start(out=h4T_t[:], in_=h4T[:, t * 128:(t + 1) * 128])
                mm1_ps = psum.tile([128, D], F32, tag="dmm")
                nc.tensor.matmul(mm1_ps[:], h4T_t[:], fc1a_t[:], start=True, stop=True)
                qq_exp = fin.tile([128, D], F32, tag="qqexp")
                gi = nc.gpsimd.indirect_dma_start(
                    out=qq_exp[:],
                    out_offset=None,
                    in_=qq_dram[:],
                    in_offset=bass.IndirectOffsetOnAxis(ap=batch_t[:, t:t + 1], axis=0),
                )
                u1 = fin.tile([128, D], F32, tag="elu_u1")
                nc.vector.tensor_add(u1[:], mm1_ps[:], qq_exp[:])
                m = fin.tile([128, D], F32, tag="elu_m")
                nc.vector.tensor_scalar(m[:], u1[:], 1.0, None, mybir.AluOpType.min)
                e = fin.tile([128, D], F32, tag="elu_e")
                nc.scalar.activation(e[:], m[:], mybir.ActivationFunctionType.Exp, bias=neg1[:, :1])
                v = fin.tile([128, D], F32, tag="elu_v")
                nc.vector.tensor_tensor(v[:], u1[:], e[:], op=mybir.AluOpType.max)
                o1 = fin.tile([128, D], F32, tag="elu_h")
                nc.scalar.activation(o1[:], v[:], mybir.ActivationFunctionType.Copy, bias=-1.0)
                tp_ps = psum.tile([128, 128], F32, tag="tp")
                nc.tensor.transpose(tp_ps[:], o1[:], ident_t[:])
                o1T = fin.tile([128, 128], F32, tag="hdT_out")
                nc.scalar.copy(o1T[:], tp_ps[:])
                mm2_ps = psum.tile([128, D], F32, tag="dmm")
                nc.tensor.matmul(mm2_ps[:], o1T[:], fc2w_t[:], start=True, stop=True)
                o2 = fin.tile([128, D], F32, tag="out2")
                nc.vector.tensor_add(o2[:], mm2_ps[:], fc2b_t[:])
                nc.sync.dma_start(out=out[t * 128:(t + 1) * 128, :], in_=o2[:])

    nc.compile()
    return nc


def _get_compiled(inputs):
    in_maps, cpt, TC = _preprocess(**inputs)
    import os
    key = ("v1", os.environ.get("NT_DEBUG", ""), os.environ.get("LAYER_REPS", ""), os.environ.get("NO_AG", ""), os.environ.get("NO_GATHER", ""), TC, tuple(cpt))
    if key not in _CACHE:
        _CACHE[key] = _build(cpt, TC)
    return _CACHE[key], in_maps


def kernel(**inputs) -> np.ndarray:
    from concourse.bass_utils import run_bass_kernel_spmd

    nc, in_maps = _get_compiled(inputs)
    res = run_bass_kernel_spmd(nc, in_maps, core_ids=list(range(NC)))
    out = np.concatenate([res.results[c]["out"][:NPC] for c in range(NC)], axis=0)
    return out.astype(np.float32)


if __name__ == "__main__":
    import sys
    sys.path.insert(0, "/root/problem")
    import reference
    inputs = {k: np.asarray(v) for k, v in reference.setup_inputs().items()}
    expected = np.asarray(reference.reference(**inputs))
    actual = kernel(**inputs)
    aerr = np.abs(actual - expected)
    denom = np.abs(expected).max()
    print("max abs err:", aerr.max(), "scale:", denom)
    print("rel err:", aerr.max() / denom)



# revision 9
# speedup vs baseline: 1.1282x; 1.1282x over previous
"""GCN (4-layer, PyG GCNConv semantics) on 8 Trainium2 NeuronCores.

Sharding: nodes partitioned into 8 contiguous blocks (graph-parallel); edges
assigned to the core owning their dst. Each layer: local dense matmul
g=(h*dis)@W, AllGather of g across cores, then per-core aggregation:
edges are bucketed by source shard-pair (4 buckets of 25088 rows, so row ids
fit in int16) and gathered in large batches with dma_gather (one SWDGE call
per (tile-group, bucket) instead of one indirect DMA per 128 edges), then
segment-summed via one-hot selection matmuls accumulating in PSUM. The
one-hot matrices are built with one batched is_equal per (tile-group, bucket)
using 3D broadcast access patterns.

Self-contained: hardcodes the problem shapes (N=100000, E=1600000, B=64, D=128).
"""

import numpy as np

import concourse.bacc as bacc
import concourse.bass as bass
import concourse.mybir as mybir
import concourse.tile as tile

N = 100000
E = 1600000
B = 64
D = 128
NC = 8
NPC = N // NC          # real nodes per core (12500)
NT = (NPC + 127) // 128  # node tiles per core (98)
NB = NT * 128          # padded node slots per core (12544)
NBUCK = 4              # src shard-pairs; bucket rows = 2*NB = 25088 < 2^15
BROWS = 2 * NB
GT = 4                 # tiles per aggregation group
NG = (NT + GT - 1) // GT  # groups (25)

F32 = mybir.dt.float32
I32 = mybir.dt.int32
I16 = mybir.dt.int16
GDT = mybir.dt.float16
GNP = np.float16

_CACHE = {}


def _preprocess(x, edge_index, batch, question_embedding,
                w0, b0, w1, b1, w2, b2, w3, b3,
                fc0_w, fc0_b, fc1_w, fc1_b, fc2_w, fc2_b):
    src = np.asarray(edge_index[0], dtype=np.int64)
    dst = np.asarray(edge_index[1], dtype=np.int64)
    x = np.asarray(x, dtype=np.float32)
    batch = np.asarray(batch, dtype=np.int64)

    deg = (np.bincount(dst, minlength=N) + 1).astype(np.float32)
    dis = deg ** -0.5

    core = dst // NPC
    rem = dst - core * NPC
    tile_of = rem >> 7                      # dst tile within core
    slot = rem & 127                        # dst slot within tile
    src_core = src // NPC
    buck = src_core >> 1                    # source bucket (shard pair)
    src_rel = ((src_core & 1) * NB + (src % NPC)).astype(np.int64)  # row in bucket

    # sort edges by (dst core, tile, bucket)
    key = (core * NT + tile_of) * NBUCK + buck
    order = np.argsort(key, kind="stable")
    srcrel_s = src_rel[order]
    slot_s = slot[order]

    cnt = np.bincount(key, minlength=NC * NT * NBUCK)  # edges per (core,tile,bucket)
    nch = np.ceil(cnt / 128).astype(np.int64)          # chunks per (core,tile,bucket)
    PAD_REL = NPC  # first pad slot of the even shard of each bucket: g row is 0

    starts = np.zeros(NC * NT * NBUCK, dtype=np.int64)
    starts[1:] = np.cumsum(cnt)[:-1]

    # Use the max chunk count across cores per (tile,bucket) so all cores
    # share one compiled SPMD kernel (same static loop structure).
    nch_tb = np.maximum(nch.reshape(NC, NT, NBUCK).max(axis=0), 1)  # [NT, NBUCK]
    meta = []           # meta[g][k] = list of chunk counts per tile in group
    for g in range(NG):
        tiles = range(g * GT, min((g + 1) * GT, NT))
        meta.append([[int(nch_tb[t, k]) for t in tiles] for k in range(NBUCK)])
    nch_gk = [[sum(meta[g][k]) for k in range(NBUCK)] for g in range(NG)]
    TCH = sum(sum(r) for r in nch_gk)       # total chunks
    ICOLS = TCH * 8                          # idx cols (int16): chunks*128/16

    idx16 = np.zeros((NC, 128, ICOLS), dtype=np.int16)
    dstf = np.zeros((NC, 128, TCH), dtype=np.float32)

    for c in range(NC):
        col = 0   # chunk column
        for g in range(NG):
            tiles = list(range(g * GT, min((g + 1) * GT, NT)))
            for k in range(NBUCK):
                seq_idx = []
                seq_dst = []
                for t in tiles:
                    kk = (c * NT + t) * NBUCK + k
                    n_e = int(cnt[kk])
                    s0 = int(starts[kk])
                    n_pad = int(nch_tb[t, k]) * 128
                    e_idx = np.full(n_pad, PAD_REL, dtype=np.int64)
                    e_dst = np.zeros(n_pad, dtype=np.float32)
                    e_idx[:n_e] = srcrel_s[s0:s0 + n_e]
                    e_dst[:n_e] = slot_s[s0:s0 + n_e]
                    seq_idx.append(e_idx)
                    seq_dst.append(e_dst)
                seq_idx = np.concatenate(seq_idx)
                seq_dst = np.concatenate(seq_dst)
                ncol = len(seq_idx) // 16
                blk = seq_idx.reshape(ncol, 16).T.astype(np.int16)
                idx16[c, :, col * 8: col * 8 + ncol] = np.tile(blk, (8, 1))
                nchk = len(seq_idx) // 128
                dstf[c, :, col: col + nchk] = seq_dst.reshape(nchk, 128).T
                col += nchk

    # per-core padded node data
    dis_pad = np.zeros((NC, NB), dtype=np.float32)
    batch_pad = np.zeros((NC, NB), dtype=np.int32)
    xds = x * dis[:, None]
    xdT = np.zeros((NC, 128, NB), dtype=np.float32)
    for c in range(NC):
        blk = slice(c * NPC, (c + 1) * NPC)
        dis_pad[c, :NPC] = dis[blk]
        batch_pad[c, :NPC] = batch[blk]
        xdT[c, :, :NPC] = xds[blk].T
    discolT = dis_pad.reshape(NC, NT, 128).transpose(0, 2, 1).copy()   # [NC,128,NT]
    batchT = batch_pad.reshape(NC, NT, 128).transpose(0, 2, 1).copy()  # [NC,128,NT]

    # weights (replicated)
    w_all = np.concatenate([np.asarray(w, np.float32) for w in (w0, w1, w2, w3)], axis=0)
    brep1 = np.concatenate(
        [np.tile(np.asarray(b, np.float32)[None, :] + 1.0, (128, 1)) for b in (b0, b1, b2, b3)],
        axis=0,
    )
    fc0_w = np.asarray(fc0_w, np.float32)
    fc0w_packed = np.concatenate([fc0_w[j * 128:(j + 1) * 128, :] for j in range(6)], axis=1)
    qT = np.asarray(question_embedding, np.float32).T
    qembT_packed = np.concatenate([qT[j * 128:(j + 1) * 128, :] for j in range(6)], axis=1)
    fc0_b1_rep = np.tile(np.asarray(fc0_b, np.float32)[None, :] + 1.0, (128, 1))
    fc1a = np.ascontiguousarray(np.asarray(fc1_w, np.float32)[:128, :])
    fc1b = np.ascontiguousarray(np.asarray(fc1_w, np.float32)[128:, :])
    fc1_b1_rep = np.tile(np.asarray(fc1_b, np.float32)[None, :] + 1.0, (128, 1))
    fc2_w = np.asarray(fc2_w, np.float32)
    fc2_b_rep = np.tile(np.asarray(fc2_b, np.float32)[None, :], (128, 1))
    iota = np.tile(np.arange(128, dtype=np.float32)[None, :], (128, 1))
    ident = np.eye(128, dtype=np.float32)
    ident_g = np.eye(128, dtype=GNP)

    shared = dict(
        w_all=w_all, brep1=brep1, fc0w_packed=fc0w_packed, qembT_packed=qembT_packed,
        fc0_b1_rep=fc0_b1_rep, fc1a=fc1a, fc1b=fc1b, fc1_b1_rep=fc1_b1_rep,
        fc2_w=fc2_w, fc2_b_rep=fc2_b_rep, iota=iota, ident=ident, ident_g=ident_g,
    )
    in_maps = []
    for c in range(NC):
        m = dict(shared)
        m.update(
            xdT=np.ascontiguousarray(xdT[c]),
            discolT=np.ascontiguousarray(discolT[c]),
            batchT=np.ascontiguousarray(batchT[c]),
            idx16=np.ascontiguousarray(idx16[c]),
            dstf=np.ascontiguousarray(dstf[c]),
        )
        in_maps.append(m)
    return in_maps, meta


def _elu_chain(nc, sbuf, psum_or_sbuf, dis_scale, brep1_t, neg1, extra_add=None):
    t = sbuf.tile([128, D], F32, tag="elu_t")
    nc.scalar.activation(t[:], psum_or_sbuf, mybir.ActivationFunctionType.Copy, scale=dis_scale)
    u1 = sbuf.tile([128, D], F32, tag="elu_u1")
    nc.vector.tensor_add(u1[:], t[:], brep1_t)
    m = sbuf.tile([128, D], F32, tag="elu_m")
    nc.vector.tensor_scalar(m[:], u1[:], 1.0, None, mybir.AluOpType.min)
    e = sbuf.tile([128, D], F32, tag="elu_e")
    nc.scalar.activation(e[:], m[:], mybir.ActivationFunctionType.Exp, bias=neg1)
    v = sbuf.tile([128, D], F32, tag="elu_v")
    nc.vector.tensor_tensor(v[:], u1[:], e[:], op=mybir.AluOpType.max)
    h = sbuf.tile([128, D], F32, tag="elu_h")
    nc.scalar.activation(h[:], v[:], mybir.ActivationFunctionType.Copy, bias=-1.0)
    if extra_add is not None:
        h2 = sbuf.tile([128, D], F32, tag="elu_h2")
        nc.vector.tensor_add(h2[:], h[:], extra_add)
        return h2
    return h


def _build(meta):
    import os
    layer_reps = int(os.environ.get("LAYER_REPS", 1))
    no_ag = bool(int(os.environ.get("NO_AG", "0")))
    no_gather = bool(int(os.environ.get("NO_GATHER", "0")))
    nch_gk = [[sum(meta[g][k]) for k in range(NBUCK)] for g in range(len(meta))]
    TCH = sum(sum(r) for r in nch_gk)
    ICOLS = TCH * 8
    nc = bacc.Bacc("TRN2", target_bir_lowering=False, debug=False, num_devices=NC)

    # per-core inputs
    xdT = nc.dram_tensor("xdT", [128, NB], F32, kind="ExternalInput")
    discolT = nc.dram_tensor("discolT", [128, NT], F32, kind="ExternalInput")
    batchT = nc.dram_tensor("batchT", [128, NT], I32, kind="ExternalInput")
    idx16_d = nc.dram_tensor("idx16", [128, ICOLS], I16, kind="ExternalInput")
    dstf_d = nc.dram_tensor("dstf", [128, TCH], F32, kind="ExternalInput")
    # replicated inputs
    w_all = nc.dram_tensor("w_all", [512, D], F32, kind="ExternalInput")
    brep1 = nc.dram_tensor("brep1", [512, D], F32, kind="ExternalInput")
    fc0w_packed = nc.dram_tensor("fc0w_packed", [128, 768], F32, kind="ExternalInput")
    qembT_packed = nc.dram_tensor("qembT_packed", [128, 384], F32, kind="ExternalInput")
    fc0_b1_rep = nc.dram_tensor("fc0_b1_rep", [128, D], F32, kind="ExternalInput")
    fc1a = nc.dram_tensor("fc1a", [128, D], F32, kind="ExternalInput")
    fc1b = nc.dram_tensor("fc1b", [128, D], F32, kind="ExternalInput")
    fc1_b1_rep = nc.dram_tensor("fc1_b1_rep", [128, D], F32, kind="ExternalInput")
    fc2_w = nc.dram_tensor("fc2_w", [128, D], F32, kind="ExternalInput")
    fc2_b_rep = nc.dram_tensor("fc2_b_rep", [128, D], F32, kind="ExternalInput")
    iota_in = nc.dram_tensor("iota", [128, 128], F32, kind="ExternalInput")
    ident_in = nc.dram_tensor("ident", [128, 128], F32, kind="ExternalInput")
    ident_g_in = nc.dram_tensor("ident_g", [128, 128], GDT, kind="ExternalInput")

    out = nc.dram_tensor("out", [NB, D], F32, kind="ExternalOutput")

    with tile.TileContext(nc) as tc:
        with (
            tc.tile_pool(name="const", bufs=1) as cpool,
            tc.tile_pool(name="wpool", bufs=2) as wpool,
            tc.tile_pool(name="dense", bufs=4) as dense,
            tc.tile_pool(name="gather", bufs=2) as gpool,
            tc.tile_pool(name="sel", bufs=2) as selpool,
            tc.tile_pool(name="meta", bufs=2) as mpool,
            tc.tile_pool(name="fin", bufs=3) as fin,
            tc.tile_pool(name="psum", bufs=2, space="PSUM") as psum,
            tc.tile_pool(name="apsum", bufs=3, space="PSUM") as apsum,
            tc.tile_pool(name="dram", bufs=1, space="DRAM") as dram,
        ):
            ag_in = dram.tile([NB, D], GDT, tag="ag_in")
            g_fulls = [dram.tile([NC * NB, D], GDT, addr_space="Shared",
                                 tag=f"g_full{l}", name=f"g_full{l}")
                       for l in range(4 * layer_reps)]
            hdT_a = dram.tile([128, NB], F32, tag="hdT_a")
            hdT_b = dram.tile([128, NB], F32, tag="hdT_b")
            h2_buf = dram.tile([NB, D], F32, tag="h2_buf")
            qq_dram = dram.tile([B, D], F32, tag="qq_dram")

            # constants
            iota_t = cpool.tile([128, 128], F32)
            nc.sync.dma_start(out=iota_t[:], in_=iota_in[:, :])
            ident_t = cpool.tile([128, 128], F32)
            nc.sync.dma_start(out=ident_t[:], in_=ident_in[:, :])
            identg_t = cpool.tile([128, 128], GDT)
            nc.sync.dma_start(out=identg_t[:], in_=ident_g_in[:, :])
            neg1 = cpool.tile([128, 1], F32)
            nc.vector.memset(neg1[:], -1.0)
            dis_t = cpool.tile([128, NT], F32)
            nc.sync.dma_start(out=dis_t[:], in_=discolT[:, :])
            batch_t = cpool.tile([128, NT], I32)
            nc.sync.dma_start(out=batch_t[:], in_=batchT[:, :])

            # ---------------- question head (replicated) ----------------
            qembT_t = cpool.tile([128, 384], F32)
            nc.sync.dma_start(out=qembT_t[:], in_=qembT_packed[:, :])
            fc0w_t = cpool.tile([128, 768], F32)
            nc.sync.dma_start(out=fc0w_t[:], in_=fc0w_packed[:, :])
            fc0b1_t = cpool.tile([128, D], F32)
            nc.sync.dma_start(out=fc0b1_t[:], in_=fc0_b1_rep[:, :])
            fc1b_t = cpool.tile([128, D], F32)
            nc.sync.dma_start(out=fc1b_t[:], in_=fc1b[:, :])
            fc1b1_t = cpool.tile([128, D], F32)
            nc.sync.dma_start(out=fc1b1_t[:], in_=fc1_b1_rep[:, :])

            q0_ps = psum.tile([64, D], F32, tag="dmm")
            for j in range(6):
                nc.tensor.matmul(
                    q0_ps[:], qembT_t[:, j * 64:(j + 1) * 64], fc0w_t[:, j * 128:(j + 1) * 128],
                    start=(j == 0), stop=(j == 5),
                )
            qu1 = cpool.tile([64, D], F32)
            nc.vector.tensor_add(qu1[:], q0_ps[:], fc0b1_t[:64, :])
            qm = cpool.tile([64, D], F32)
            nc.vector.tensor_scalar(qm[:], qu1[:], 1.0, None, mybir.AluOpType.min)
            qe = cpool.tile([64, D], F32)
            nc.scalar.activation(qe[:], qm[:], mybir.ActivationFunctionType.Exp, bias=neg1[:64, :1])
            qv = cpool.tile([64, D], F32)
            nc.vector.tensor_tensor(qv[:], qu1[:], qe[:], op=mybir.AluOpType.max)
            q_t = cpool.tile([64, D], F32)
            nc.scalar.activation(q_t[:], qv[:], mybir.ActivationFunctionType.Copy, bias=-1.0)
            qT_ps = psum.tile([128, 64], F32, tag="tp")
            nc.tensor.transpose(qT_ps[:], q_t[:], ident_t[:64, :64])
            qT_t = cpool.tile([128, 64], F32)
            nc.vector.tensor_copy(qT_t[:], qT_ps[:])
            qq_ps = psum.tile([64, D], F32, tag="dmm")
            nc.tensor.matmul(qq_ps[:], qT_t[:], fc1b_t[:], start=True, stop=True)
            qq_t = cpool.tile([64, D], F32)
            nc.vector.tensor_add(qq_t[:], qq_ps[:], fc1b1_t[:64, :])
            nc.sync.dma_start(out=qq_dram[:, :], in_=qq_t[:])

            # ---------------- GCN layers ----------------
            dense_src = [xdT, hdT_a, hdT_b, hdT_a] * layer_reps
            agg_dstT = [hdT_a, hdT_b, hdT_a, hdT_b] * layer_reps
            for rep in range(layer_reps - 1):
                dense_src[4 * (rep + 1)] = hdT_b
            for glayer in range(4 * layer_reps):
                layer = glayer % 4
                w_t = wpool.tile([128, D], F32, tag="w")
                nc.sync.dma_start(out=w_t[:], in_=w_all[layer * 128:(layer + 1) * 128, :])
                b1_t = wpool.tile([128, D], F32, tag="b")
                nc.sync.dma_start(out=b1_t[:], in_=brep1[layer * 128:(layer + 1) * 128, :])

                # dense: g = (h*dis) @ w -> ag_in (GDT)
                hsrc = dense_src[glayer]
                for t in range(NT):
                    hdT_t = dense.tile([128, 128], F32, tag="hdT_in")
                    nc.sync.dma_start(out=hdT_t[:], in_=hsrc[:, t * 128:(t + 1) * 128])
                    g_ps = psum.tile([128, D], F32, tag="dmm")
                    nc.tensor.matmul(g_ps[:], hdT_t[:], w_t[:], start=True, stop=True)
                    g_sb = dense.tile([128, D], GDT, tag="g_out")
                    nc.scalar.copy(g_sb[:], g_ps[:])
                    nc.sync.dma_start(out=ag_in[t * 128:(t + 1) * 128, :], in_=g_sb[:])

                g_full = g_fulls[glayer]
                if not no_ag:
                    nc.gpsimd.collective_compute(
                        "AllGather",
                        mybir.AluOpType.bypass,
                        replica_groups=[list(range(NC))],
                        ins=[ag_in.opt()],
                        outs=[g_full.opt()],
                    )

                # aggregation: groups of GT tiles, 4 source buckets each
                hdst = agg_dstT[glayer]
                col = 0
                for g in range(len(meta)):
                    tiles = list(range(g * GT, min((g + 1) * GT, NT)))
                    gbufs, sels = [], []
                    for k in range(NBUCK):
                        nchk = nch_gk[g][k]
                        idx_t = mpool.tile([128, nchk * 8], I16, tag=f"idx{k}")
                        nc.sync.dma_start(out=idx_t[:], in_=idx16_d[:, col * 8:(col + nchk) * 8])
                        dstf_t = mpool.tile([128, nchk], F32, tag=f"dstf{k}")
                        nc.sync.dma_start(out=dstf_t[:], in_=dstf_d[:, col:col + nchk])
                        gb = gpool.tile([128, nchk * D], GDT, tag=f"gbuf{k}")
                        if not no_gather:
                            nc.gpsimd.dma_gather(
                                out_ap=gb[:].rearrange("p (c d) -> p c d", d=D),
                                in_ap=g_full[k * BROWS:(k + 1) * BROWS, :],
                                idxs_ap=idx_t[:],
                                num_idxs=nchk * 128,
                                num_idxs_reg=nchk * 128,
                                elem_size=D,
                                single_packet=False,
                            )
                        sel = selpool.tile([128, nchk * 128], GDT, tag=f"sel{k}")
                        nc.vector.tensor_tensor(
                            sel[:].rearrange("p (c j) -> p c j", j=128),
                            dstf_t[:].unsqueeze(2).broadcast_to([128, nchk, 128]),
                            iota_t[:].unsqueeze(1).broadcast_to([128, nchk, 128]),
                            op=mybir.AluOpType.is_equal,
                        )
                        gbufs.append(gb)
                        sels.append(sel)
                        col += nchk

                    for ti, t in enumerate(tiles):
                        agg_ps = apsum.tile([128, D], F32, tag="agg")
                        for k in range(NBUCK):
                            coff = sum(meta[g][k][:ti])
                            for c in range(meta[g][k][ti]):
                                j = coff + c
                                nc.tensor.matmul(
                                    agg_ps[:],
                                    sels[k][:, j * 128:(j + 1) * 128],
                                    gbufs[k][:, j * D:(j + 1) * D],
                                    start=(k == 0 and c == 0),
                                    stop=False,
                                )
                        gself = fin.tile([128, D], GDT, tag="gself")
                        nc.sync.dma_start(out=gself[:], in_=ag_in[t * 128:(t + 1) * 128, :])
                        nc.tensor.matmul(agg_ps[:], identg_t[:], gself[:], start=False, stop=True)

                        extra = None
                        if layer == 3:
                            extra_t = fin.tile([128, D], F32, tag="h2in")
                            nc.sync.dma_start(out=extra_t[:], in_=h2_buf[t * 128:(t + 1) * 128, :])
                            extra = extra_t[:]
                        h_t = _elu_chain(nc, fin, agg_ps[:], dis_t[:, t:t + 1], b1_t[:], neg1[:, :1], extra_add=extra)

                        if layer == 1:
                            nc.sync.dma_start(out=h2_buf[t * 128:(t + 1) * 128, :], in_=h_t[:])
                        if layer < 3:
                            hd = fin.tile([128, D], F32, tag="hd")
                            nc.vector.tensor_scalar(
                                hd[:], h_t[:], dis_t[:, t:t + 1], None, mybir.AluOpType.mult)
                            tp_ps = psum.tile([128, 128], F32, tag="tp")
                            nc.tensor.transpose(tp_ps[:], hd[:], ident_t[:])
                        else:
                            tp_ps = psum.tile([128, 128], F32, tag="tp")
                            nc.tensor.transpose(tp_ps[:], h_t[:], ident_t[:])
                        hdT_o = fin.tile([128, 128], F32, tag="hdT_out")
                        nc.scalar.copy(hdT_o[:], tp_ps[:])
                        nc.sync.dma_start(out=hdst[:, t * 128:(t + 1) * 128], in_=hdT_o[:])
                assert col == TCH, (col, TCH)

            # ---------------- MLP head ----------------
            fc1a_t = cpool.tile([128, D], F32)
            nc.sync.dma_start(out=fc1a_t[:], in_=fc1a[:, :])
            fc2w_t = cpool.tile([128, D], F32)
            nc.sync.dma_start(out=fc2w_t[:], in_=fc2_w[:, :])
            fc2b_t = cpool.tile([128, D], F32)
            nc.sync.dma_start(out=fc2b_t[:], in_=fc2_b_rep[:, :])

            h4T = agg_dstT[3]
            for t in range(NT):
                h4T_t = dense.tile([128, 128], F32, tag="hdT_in")
                nc.sync.dma_start(out=h4T_t[:], in_=h4T[:, t * 128:(t + 1) * 128])
                mm1_ps = psum.tile([128, D], F32, tag="dmm")
                nc.tensor.matmul(mm1_ps[:], h4T_t[:], fc1a_t[:], start=True, stop=True)
                qq_exp = fin.tile([128, D], F32, tag="qqexp")
                nc.gpsimd.indirect_dma_start(
                    out=qq_exp[:],
                    out_offset=None,
                    in_=qq_dram[:],
                    in_offset=bass.IndirectOffsetOnAxis(ap=batch_t[:, t:t + 1], axis=0),
                )
                u1 = fin.tile([128, D], F32, tag="elu_u1")
                nc.vector.tensor_add(u1[:], mm1_ps[:], qq_exp[:])
                m = fin.tile([128, D], F32, tag="elu_m")
                nc.vector.tensor_scalar(m[:], u1[:], 1.0, None, mybir.AluOpType.min)
                e = fin.tile([128, D], F32, tag="elu_e")
                nc.scalar.activation(e[:], m[:], mybir.ActivationFunctionType.Exp, bias=neg1[:, :1])
                v = fin.tile([128, D], F32, tag="elu_v")
                nc.vector.tensor_tensor(v[:], u1[:], e[:], op=mybir.AluOpType.max)
                o1 = fin.tile([128, D], F32, tag="elu_h")
                nc.scalar.activation(o1[:], v[:], mybir.ActivationFunctionType.Copy, bias=-1.0)
                tp_ps = psum.tile([128, 128], F32, tag="tp")
                nc.tensor.transpose(tp_ps[:], o1[:], ident_t[:])
                o1T = fin.tile([128, 128], F32, tag="hdT_out")
                nc.scalar.copy(o1T[:], tp_ps[:])
                mm2_ps = psum.tile([128, D], F32, tag="dmm")
                nc.tensor.matmul(mm2_ps[:], o1T[:], fc2w_t[:], start=True, stop=True)
                o2 = fin.tile([128, D], F32, tag="out2")
                nc.vector.tensor_add(o2[:], mm2_ps[:], fc2b_t[:])
                nc.sync.dma_start(out=out[t * 128:(t + 1) * 128, :], in_=o2[:])

    nc.compile()
    return nc


def _meta_key(meta):
    return tuple(tuple(tuple(r) for r in g) for g in meta)


def _get_compiled(inputs):
    in_maps, meta = _preprocess(**inputs)
    import os
    key = ("v4", os.environ.get("LAYER_REPS", ""), os.environ.get("NO_AG", ""),
           os.environ.get("NO_GATHER", ""), _meta_key(meta))
    if key not in _CACHE:
        _CACHE[key] = _build(meta)
    return _CACHE[key], in_maps


def kernel(**inputs) -> np.ndarray:
    from concourse.bass_utils import run_bass_kernel_spmd

    nc, in_maps = _get_compiled(inputs)
    res = run_bass_kernel_spmd(nc, in_maps, core_ids=list(range(NC)))
    out = np.concatenate([res.results[c]["out"][:NPC] for c in range(NC)], axis=0)
    return out.astype(np.float32)


if __name__ == "__main__":
    import sys
    sys.path.insert(0, "/root/problem")
    data = np.load("/tmp/ref_data.npz")
    expected = data["expected"]
    inputs = {k: data[k] for k in data.files if k != "expected"}
    actual = kernel(**inputs)
    aerr = np.abs(actual - expected)
    denom = np.abs(expected).max()
    print("max abs err:", aerr.max(), "scale:", denom)
    print("rel err:", aerr.max() / denom)
